# revision 1
# baseline (speedup 1.0000x reference)
"""4-bit comparator (a>b, a==b) over [8388608, 4] binary spike inputs.

Strategy: rows are data-parallel across 8 NeuronCores. On each core the
integer values of the 4-bit operands are compared via their weighted
difference d = sum_j w_j*(A_j - B_j), w = [8,4,2,1] (MSB first), computed
entirely on the TensorEngine as 8 accumulating matmuls with scaled-identity
stationary weights (+w_j*I for A, -w_j*I for B) over stride-4 free slices
of the natural-layout tiles. PSUM then holds the exact integer difference
in f32; DVE emits is_gt(d,0) and is_equal(d,0) as bf16 {0,1}.

Inputs are {0.0, 1.0} so a host-side cast to bf16 is exact and halves HBM
traffic; outputs travel back as bf16 {0,1} and are expanded to f32 on host.
"""

import sys

if "/opt/trn_rl_repo" not in sys.path:
    sys.path.insert(0, "/opt/trn_rl_repo")

import numpy as np
import ml_dtypes

N_ROWS = 8_388_608
N_CORES = 8
R = N_ROWS // N_CORES          # rows per core = 1,048,576
P = 128                        # SBUF partitions
EPP = R * 4 // P               # elements per partition per input = 32768
MPP = R // P                   # rows (groups) per partition = 8192
CH = 4096                      # input elems per partition per chunk (1MB DMA)
NCH = EPP // CH                # 8 chunks
MT = 512                       # psum free size (one bank)
W_BITS = (8.0, 4.0, 2.0, 1.0)  # MSB-first bit weights

_CACHE = {}


def _build(reps=1):
    import concourse.bass as bass
    import concourse.mybir as mybir

    nc = bass.Bass(trn_type="TRN2")
    bf16 = mybir.dt.bfloat16
    f32 = mybir.dt.float32
    A = nc.dram_tensor("A", [P, EPP], bf16, kind="ExternalInput")
    B = nc.dram_tensor("B", [P, EPP], bf16, kind="ExternalInput")
    out = nc.dram_tensor("out", [2, P, MPP], bf16, kind="ExternalOutput")

    # 8 stationary weights: [pin, k, po] = w_k * I for A slices, -w_k * I for B
    wnp = np.zeros((P, 8, P), dtype=ml_dtypes.bfloat16)
    for k in range(4):
        for p in range(P):
            wnp[p, k, p] = W_BITS[k]
            wnp[p, 4 + k, p] = -W_BITS[k]
    wdram = nc.inline_tensor(wnp, name="wconst")

    NG = 2 * NCH               # psum groups per core (16)
    m_ch = CH // 4             # groups-of-4 per chunk (1024)
    AluOp = mybir.AluOpType

    from contextlib import ExitStack
    with ExitStack() as ctx:
        ec = ctx.enter_context
        wt = ec(nc.sbuf_tensor("wt", [P, 8, P], bf16))
        at = [ec(nc.sbuf_tensor(f"at{i}", [P, CH], bf16)) for i in range(3)]
        bt = [ec(nc.sbuf_tensor(f"bt{i}", [P, CH], bf16)) for i in range(3)]
        gts = [ec(nc.sbuf_tensor(f"gt{i}", [P, MT], bf16)) for i in range(3)]
        eqs = [ec(nc.sbuf_tensor(f"eq{i}", [P, MT], bf16)) for i in range(3)]
        pss = [ec(nc.psum_tensor(f"ps{i}", [P, MT], f32)) for i in range(4)]
        s_w = ec(nc.semaphore(name="s_w"))
        s_in = [ec(nc.semaphore(name=f"s_in{i}")) for i in range(3)]
        s_peg = ec(nc.semaphore(name="s_peg"))
        s_cmp = ec(nc.semaphore(name="s_cmp"))
        s_out = [ec(nc.semaphore(name=f"s_out{i}")) for i in range(3)]
        block = ec(nc.Block())
        NCT = reps * NCH           # total chunk iterations
        NGT = 2 * NCT              # total psum groups
        # out-DMA count (×16) per rotating slot j: groups g ≡ j (mod 3)
        outs_per_slot = [2 * len([g for g in range(NGT) if g % 3 == j])
                         for j in range(3)]

        @block.sync
        def _(sync):
            sync.dma_start(wt[:], wdram[:]).then_inc(s_w, 16)
            for cc in range(NCT):
                if cc >= 3:
                    # chunk cc-3's matmuls (2 groups each inc s_peg) done
                    sync.wait_ge(s_peg, 2 * (cc - 2))
                c = cc % NCH
                sl = slice(c * CH, (c + 1) * CH)
                sync.dma_start(at[cc % 3][:], A[:, sl]).then_inc(s_in[cc % 3], 16)
                sync.dma_start(bt[cc % 3][:], B[:, sl]).then_inc(s_in[cc % 3], 16)
            for j in range(3):
                sync.wait_ge(s_out[j], 16 * outs_per_slot[j])

        @block.tensor
        def _(pe):
            pe.wait_ge(s_w, 16)
            for cc in range(NCT):
                pe.wait_ge(s_in[cc % 3], 32 * (cc // 3 + 1))
                av = at[cc % 3][:].rearrange("p (m k) -> p k m", k=4)
                bv = bt[cc % 3][:].rearrange("p (m k) -> p k m", k=4)
                for h in range(2):
                    g = 2 * cc + h
                    if g >= 4:
                        # psum slot g%4 reused from group g-4: its compares done
                        pe.wait_ge(s_cmp, 2 * (g - 4) + 2)
                    sl = slice(h * MT, (h + 1) * MT)
                    mm = None
                    for ki in range(8):
                        src = av if ki < 4 else bv
                        mm = nc.tensor.matmul(
                            pss[g % 4][:],
                            wt[:, ki, :],
                            src[:, ki % 4, sl],
                            start=(ki == 0),
                            stop=(ki == 7),
                        )
                    mm.then_inc(s_peg, 1)

        @block.vector
        def _(dve):
            for g in range(NGT):
                dve.wait_ge(s_peg, g + 1)
                if g >= 3:
                    # gt/eq slot g%3 reused from group g-3: its out-DMAs done
                    dve.wait_ge(s_out[g % 3], 32 * (g // 3))
                nc.vector.tensor_scalar(
                    out=gts[g % 3][:], in0=pss[g % 4][:],
                    scalar1=0.0, scalar2=None, op0=AluOp.is_gt,
                ).then_inc(s_cmp, 1)
                nc.vector.tensor_scalar(
                    out=eqs[g % 3][:], in0=pss[g % 4][:],
                    scalar1=0.0, scalar2=None, op0=AluOp.is_equal,
                ).then_inc(s_cmp, 1)

        @block.scalar
        def _(act):
            for g in range(NGT):
                act.wait_ge(s_cmp, 2 * (g + 1))
                gg = g % NG
                c, h = gg // 2, gg % 2
                osl = slice(c * m_ch + h * MT, c * m_ch + (h + 1) * MT)
                act.dma_start(out[0, :, osl], gts[g % 3][:]).then_inc(
                    s_out[g % 3], 16)
                act.dma_start(out[1, :, osl], eqs[g % 3][:]).then_inc(
                    s_out[g % 3], 16)

    return nc


def _get_nc():
    if "nc" not in _CACHE:
        _CACHE["nc"] = _build()
    return _CACHE["nc"]


def kernel(A, B, trace=False):
    from concourse import bass_utils

    A = np.asarray(A)
    B = np.asarray(B)
    assert A.shape == (N_ROWS, 4) and B.shape == (N_ROWS, 4), (A.shape, B.shape)

    bf = ml_dtypes.bfloat16
    in_maps = []
    for i in range(N_CORES):
        sl = slice(i * R, (i + 1) * R)
        in_maps.append({
            "A": np.ascontiguousarray(A[sl]).astype(bf).reshape(P, EPP),
            "B": np.ascontiguousarray(B[sl]).astype(bf).reshape(P, EPP),
        })

    nc = _get_nc()
    res = bass_utils.run_bass_kernel_spmd(
        nc, in_maps, core_ids=list(range(N_CORES)), trace=trace,
    )
    _CACHE["last_results"] = res

    gt = np.empty((N_ROWS,), dtype=np.float32)
    eq = np.empty((N_ROWS,), dtype=np.float32)
    for i in range(N_CORES):
        o = np.asarray(res.results[i]["out"])  # [2, P, MPP] bf16
        sl = slice(i * R, (i + 1) * R)
        gt[sl] = o[0].reshape(R).astype(np.float32)
        eq[sl] = o[1].reshape(R).astype(np.float32)
    return gt.reshape(N_ROWS, 1), eq.reshape(N_ROWS, 1)



# revision 3
# speedup vs baseline: 1.7495x; 1.7495x over previous
"""4-bit comparator (a>b, a==b) over [8388608, 4] binary spike inputs.

Strategy: rows are data-parallel across 8 NeuronCores. On each core the
integer values of the 4-bit operands are compared via their weighted
difference d = sum_j w_j*(A_j - B_j), w = [8,4,2,1] (MSB first), computed
on the TensorEngine as accumulating matmuls with scaled-identity
stationary weights. Inputs are {0,1} so fp8(e4m3) holds them exactly:
the host casts f32 -> fp8 and lays each core's slice out bit-PLANAR per
chunk (4 contiguous bit planes per partition) so the PE moving access
patterns are contiguous and pairs of planes form DoubleRow k-subtiles
(fp8 perf mode, 2 MACs/cell/cycle => 4 matmuls per PSUM group).

The Scalar engine then emits o = sign(d) in {-1,0,+1} as a single fp8
byte per row (gt <=> +1, eq <=> 0), which the host decodes with two byte
compares. HBM traffic per core: 8 MiB in + 1 MiB out (vs 41.9 MiB all-f32).
"""

import os
import sys

if "/opt/trn_rl_repo" not in sys.path:
    sys.path.insert(0, "/opt/trn_rl_repo")

import numpy as np
import ml_dtypes

N_ROWS = 8_388_608
N_CORES = 8
R = N_ROWS // N_CORES          # rows per core = 1,048,576
P = 128                        # SBUF partitions
MPP = R // P                   # rows per partition = 8192
NCH = 4                        # input chunks per core
TCH = MPP // NCH               # rows per partition per chunk = 2048
CHE = 4 * TCH                  # input elems per partition per chunk = 8192
GPC = 4                        # psum groups per chunk
MT = TCH // GPC                # rows per group = 512 (one psum bank)
NG = NCH * GPC                 # total groups = 16
GPS = 4                        # groups per output slab
NSLAB = NG // GPS              # out DMAs = 4
W_BITS = (8.0, 4.0, 2.0, 1.0)  # MSB-first bit weights

DR = os.environ.get("DR", "1") == "1"   # fp8 DoubleRow perf mode

_CACHE = {}


def _build(mpp=MPP, dr=DR):
    import concourse.bass as bass
    import concourse.mybir as mybir

    nch, gpc = NCH, GPC
    tch = mpp // nch
    che = 4 * tch
    mt = tch // gpc
    ng = nch * gpc
    nslab = ng // GPS
    epp = 4 * mpp

    nc = bass.Bass(trn_type="TRN2")
    f8 = mybir.dt.float8e4
    f32 = mybir.dt.float32
    A = nc.dram_tensor("A", [P, epp], f8, kind="ExternalInput")
    B = nc.dram_tensor("B", [P, epp], f8, kind="ExternalInput")
    out = nc.dram_tensor("out", [P, mpp], f8, kind="ExternalOutput")

    # stationary weights: rows 0..3 = +w_k * I, rows 4..7 = -w_k * I
    wnp = np.zeros((P, 8, P), dtype=ml_dtypes.float8_e4m3)
    for k in range(4):
        for p in range(P):
            wnp[p, k, p] = W_BITS[k]
            wnp[p, 4 + k, p] = -W_BITS[k]
    wdram = nc.inline_tensor(wnp, name="wconst")

    from contextlib import ExitStack
    with ExitStack() as ctx:
        ec = ctx.enter_context
        wt = ec(nc.sbuf_tensor("wt", [P, 8, P], f8))
        at = [ec(nc.sbuf_tensor(f"at{i}", [P, che], f8)) for i in range(3)]
        bt = [ec(nc.sbuf_tensor(f"bt{i}", [P, che], f8)) for i in range(3)]
        ot = ec(nc.sbuf_tensor("ot", [P, mpp], f8))
        pss = [ec(nc.psum_tensor(f"ps{i}", [P, mt], f32)) for i in range(4)]
        s_w = ec(nc.semaphore(name="s_w"))
        s_in = [ec(nc.semaphore(name=f"s_in{i}")) for i in range(3)]
        s_peg = ec(nc.semaphore(name="s_peg"))
        s_cmp = ec(nc.semaphore(name="s_cmp"))
        s_out = ec(nc.semaphore(name="s_out"))
        block = ec(nc.Block())

        @block.sync
        def _(sync):
            sync.dma_start(wt[:], wdram[:]).then_inc(s_w, 16)
            for cc in range(nch):
                if cc >= 3:
                    # buffer slot cc%3 reused from chunk cc-3: PE done with it
                    sync.wait_ge(s_peg, gpc * (cc - 2))
                sl = slice(cc * che, (cc + 1) * che)
                sync.dma_start(at[cc % 3][:], A[:, sl]).then_inc(s_in[cc % 3], 16)
                sync.dma_start(bt[cc % 3][:], B[:, sl]).then_inc(s_in[cc % 3], 16)
            sync.wait_ge(s_out, 16 * nslab)

        @block.tensor
        def _(pe):
            pe.wait_ge(s_w, 16)
            for cc in range(nch):
                pe.wait_ge(s_in[cc % 3], 32 * (cc // 3 + 1))
                av = at[cc % 3][:].rearrange("p (k t) -> p k t", k=4)
                bv = bt[cc % 3][:].rearrange("p (k t) -> p k t", k=4)
                for h in range(gpc):
                    g = gpc * cc + h
                    if g >= 4:
                        # psum slot g%4 reused from group g-4: its sign done
                        pe.wait_ge(s_cmp, g - 3)
                    tsl = slice(h * mt, (h + 1) * mt)
                    if dr:
                        specs = ((av, 0, 0), (av, 2, 2), (bv, 0, 4), (bv, 2, 6))
                        for idx, (src, kp, wj) in enumerate(specs):
                            mm = nc.tensor.matmul(
                                pss[g % 4][:],
                                wt[:, wj:wj + 2, :],
                                src[:, kp:kp + 2, tsl],
                                start=(idx == 0),
                                stop=(idx == 3),
                                perf_mode=mybir.MatmulPerfMode.DoubleRow,
                            )
                    else:
                        for ki in range(8):
                            src = av if ki < 4 else bv
                            mm = nc.tensor.matmul(
                                pss[g % 4][:],
                                wt[:, ki, :],
                                src[:, ki % 4, tsl],
                                start=(ki == 0),
                                stop=(ki == 7),
                            )
                    mm.then_inc(s_peg, 1)

        @block.scalar
        def _(act):
            for g in range(ng):
                act.wait_ge(s_peg, g + 1)
                nc.scalar.sign(
                    ot[:, g * mt:(g + 1) * mt], pss[g % 4][:]
                ).then_inc(s_cmp, 1)
                if g % GPS == GPS - 1:
                    # trivially-satisfied wait: orders the async DMA read of
                    # ot behind this engine's own sign writes for the DGE
                    act.wait_ge(s_cmp, g + 1)
                    slab = slice((g + 1 - GPS) * mt, (g + 1) * mt)
                    act.dma_start(out[:, slab], ot[:, slab]).then_inc(s_out, 16)

    return nc


def _get_nc():
    if "nc" not in _CACHE:
        _CACHE["nc"] = _build()
    return _CACHE["nc"]


FP8_ONE = np.uint8(0x38)  # e4m3 +1.0


def _to_planar_fp8(x, sl):
    """Core slice [R,4] f32 {0,1} -> [P, EPP] fp8, chunk-planar layout:
    partition p, free = c*CHE + k*TCH + t  for row r = p*MPP + c*TCH + t, bit k.
    """
    b = (np.asarray(x[sl]).reshape(P, NCH, TCH, 4) != 0)
    pl = np.ascontiguousarray(b.transpose(0, 1, 3, 2)).reshape(P, 4 * MPP)
    return (pl.astype(np.uint8) * FP8_ONE).view(ml_dtypes.float8_e4m3)


def kernel(A, B, trace=False):
    from concourse import bass_utils

    A = np.asarray(A)
    B = np.asarray(B)
    assert A.shape == (N_ROWS, 4) and B.shape == (N_ROWS, 4), (A.shape, B.shape)

    in_maps = []
    for i in range(N_CORES):
        sl = slice(i * R, (i + 1) * R)
        in_maps.append({
            "A": _to_planar_fp8(A, sl),
            "B": _to_planar_fp8(B, sl),
        })

    nc = _get_nc()
    res = bass_utils.run_bass_kernel_spmd(
        nc, in_maps, core_ids=list(range(N_CORES)), trace=trace,
    )
    _CACHE["last_results"] = res

    gt = np.empty((N_ROWS,), dtype=np.float32)
    eq = np.empty((N_ROWS,), dtype=np.float32)
    for i in range(N_CORES):
        o = np.asarray(res.results[i]["out"]).view(np.uint8).reshape(R)
        sl = slice(i * R, (i + 1) * R)
        gt[sl] = (o == FP8_ONE)
        eq[sl] = ((o & 0x7F) == 0)
    return gt.reshape(N_ROWS, 1), eq.reshape(N_ROWS, 1)


# revision 7
# speedup vs baseline: 1.8869x; 1.0785x over previous
"""4-bit comparator (a>b, a==b) over [8388608, 4] binary spike inputs.

Strategy: rows are data-parallel across 8 NeuronCores. On each core the
integer values of the 4-bit operands are compared via their weighted
difference d = sum_j w_j*(A_j - B_j), w = [8,4,2,1] (MSB first), computed
on the TensorEngine as accumulating matmuls with scaled-identity
stationary weights. Inputs are {0,1} so fp8(e4m3) holds them exactly:
the host casts f32 -> fp8 and packs each core's slice into ONE tensor,
chunked bit-planar ([A planes | B planes] per 512-row chunk) so PE
moving access patterns are contiguous and plane pairs form DoubleRow
k-subtiles (fp8 perf mode: 2 MACs/cell/cycle, 4 matmuls per group).

Pipelining: 16 chunk buffers are preallocated (no reuse), even chunks
stream on the sync HWDGE ring, odd chunks + weights on the act ring, so
both rings start immediately and run concurrently toward the HBM cap.
Chunks are 512 KiB = 1 psum group, so PE idle gaps stay far below the
~3.4us HAM window and the PE clock holds 2.4 GHz; dummy warm-up matmuls
against the weight tile ramp the clock before chunk 0 lands. The Scalar
engine emits o = sign(d) in {-1,0,+1} as one fp8 byte per row (gt <=> +1,
eq <=> 0), decoded on host with two byte compares. Output slabs of 4
groups alternate rings. HBM per core: 8 MiB in + 1 MiB out.
"""

import os
import sys

if "/opt/trn_rl_repo" not in sys.path:
    sys.path.insert(0, "/opt/trn_rl_repo")

import numpy as np
import ml_dtypes

N_ROWS = 8_388_608
N_CORES = 8
R = N_ROWS // N_CORES          # rows per core = 1,048,576
P = 128                        # SBUF partitions
MPP = R // P                   # rows per partition = 8192
NCH = 16                       # chunks per core (= psum groups)
TCH = MPP // NCH               # rows per partition per chunk = 512
CHE = 8 * TCH                  # AB elems per partition per chunk = 4096
GPS = 4                        # groups per output slab
NSLAB = NCH // GPS             # out DMAs = 4
NPS = 8                        # psum banks in flight
W_BITS = (8.0, 4.0, 2.0, 1.0)  # MSB-first bit weights

DR = os.environ.get("DR", "1") == "1"   # fp8 DoubleRow perf mode
WARM = int(os.environ.get("WARM", "24"))  # PE warm-up dummy matmuls

_CACHE = {}


def _build(mpp=MPP, dr=DR):
    import concourse.bass as bass
    import concourse.mybir as mybir

    nch = NCH
    tch = mpp // nch
    che = 8 * tch
    nslab = nch // GPS

    nc = bass.Bass(trn_type="TRN2")
    f8 = mybir.dt.float8e4
    f32 = mybir.dt.float32
    AB = nc.dram_tensor("AB", [P, nch * che], f8, kind="ExternalInput")
    out = nc.dram_tensor("out", [P, mpp], f8, kind="ExternalOutput")

    # stationary weights: rows 0..3 = +w_k * I (A planes), 4..7 = -w_k * I
    wnp = np.zeros((P, 8, P), dtype=ml_dtypes.float8_e4m3)
    for k in range(4):
        for p in range(P):
            wnp[p, k, p] = W_BITS[k]
            wnp[p, 4 + k, p] = -W_BITS[k]
    wdram = nc.inline_tensor(wnp, name="wconst")

    from contextlib import ExitStack
    with ExitStack() as ctx:
        ec = ctx.enter_context
        wt = ec(nc.sbuf_tensor("wt", [P, 8, P], f8))
        cb = [ec(nc.sbuf_tensor(f"cb{i}", [P, che], f8)) for i in range(nch)]
        ot = ec(nc.sbuf_tensor("ot", [P, mpp], f8))
        pss = [ec(nc.psum_tensor(f"ps{i}", [P, tch], f32)) for i in range(NPS)]
        s_w = ec(nc.semaphore(name="s_w"))
        # 3 rotating completion sems per ring: at most one in-flight DMA per
        # sem (issuer waits before reuse), so every wait value is exact
        s_inS = [ec(nc.semaphore(name=f"s_inS{i}")) for i in range(3)]
        s_inA = [ec(nc.semaphore(name=f"s_inA{i}")) for i in range(3)]
        s_peg = ec(nc.semaphore(name="s_peg"))
        s_cmp = ec(nc.semaphore(name="s_cmp"))
        s_out = ec(nc.semaphore(name="s_out"))
        block = ec(nc.Block())

        @block.sync
        def _(sync):
            # even chunks on the sync ring, pipelined 3 deep
            for p in range(nch // 2):
                if p >= 3:
                    sync.wait_ge(s_inS[p % 3], 16 * (p // 3))
                c = 2 * p
                sl = slice(c * che, (c + 1) * che)
                sync.dma_start(cb[c][:], AB[:, sl]).then_inc(s_inS[p % 3], 16)
            # even slabs ride at the back of the sync ring
            for s in range(0, nslab, 2):
                sync.wait_ge(s_cmp, GPS * (s + 1))
                slab = slice(s * GPS * tch, (s + 1) * GPS * tch)
                sync.dma_start(out[:, slab], ot[:, slab]).then_inc(s_out, 16)
            sync.wait_ge(s_out, 16 * nslab)

        @block.scalar
        def _(act):
            # weights first, then odd chunks on the act ring, 3 deep
            act.dma_start(wt[:], wdram[:]).then_inc(s_w, 16)
            for p in range(nch // 2):
                if p >= 3:
                    act.wait_ge(s_inA[p % 3], 16 * (p // 3))
                c = 2 * p + 1
                sl = slice(c * che, (c + 1) * che)
                act.dma_start(cb[c][:], AB[:, sl]).then_inc(s_inA[p % 3], 16)
            for g in range(nch):
                act.wait_ge(s_peg, g + 1)
                nc.scalar.sign(
                    ot[:, g * tch:(g + 1) * tch], pss[g % NPS][:]
                ).then_inc(s_cmp, 1)
                s, r = divmod(g, GPS)
                if r == GPS - 1 and s % 2 == 1:
                    # trivially-satisfied: orders the DMA read behind the signs
                    act.wait_ge(s_cmp, GPS * (s + 1))
                    slab = slice(s * GPS * tch, (s + 1) * GPS * tch)
                    act.dma_start(out[:, slab], ot[:, slab]).then_inc(s_out, 16)

        @block.tensor
        def _(pe):
            pe.wait_ge(s_w, 16)
            # warm the HAM clock gate while chunk 0 is still in flight
            wfd = min(tch, P)
            for _ in range(WARM):
                nc.tensor.matmul(
                    pss[NPS - 1][:, 0:wfd],
                    wt[:, 0:2, :],
                    wt[:, 0:2, 0:wfd],
                    start=True, stop=True,
                    perf_mode=mybir.MatmulPerfMode.DoubleRow,
                    skip_group_check=True,
                )
            for g in range(nch):
                p = g // 2
                sem = s_inS if g % 2 == 0 else s_inA
                pe.wait_ge(sem[p % 3], 16 * (p // 3 + 1))
                if g >= NPS:
                    # psum slot g%NPS reused from group g-NPS: its sign done
                    pe.wait_ge(s_cmp, g - NPS + 1)
                cv = cb[g][:].rearrange("p (j t) -> p j t", j=8)
                if dr:
                    for idx in range(4):
                        mm = nc.tensor.matmul(
                            pss[g % NPS][:],
                            wt[:, 2 * idx:2 * idx + 2, :],
                            cv[:, 2 * idx:2 * idx + 2, :],
                            start=(idx == 0),
                            stop=(idx == 3),
                            perf_mode=mybir.MatmulPerfMode.DoubleRow,
                            skip_group_check=(g % NPS == NPS - 1),
                        )
                else:
                    for ki in range(8):
                        mm = nc.tensor.matmul(
                            pss[g % NPS][:],
                            wt[:, ki, :],
                            cv[:, ki, :],
                            start=(ki == 0),
                            stop=(ki == 7),
                            skip_group_check=(g % NPS == NPS - 1),
                        )
                mm.then_inc(s_peg, 1)

    return nc


def _get_nc():
    if "nc" not in _CACHE:
        _CACHE["nc"] = _build()
    return _CACHE["nc"]


FP8_ONE = np.uint8(0x38)  # e4m3 +1.0


def _pack_ab(A, B, sl, mpp=MPP):
    """Core slices [R,4] f32 {0,1} -> [P, 2*4*mpp] fp8, chunk layout:
    free = c*8*TCH + ab*4*TCH + k*TCH + t for row r = p*mpp + c*TCH + t.
    """
    tch = mpp // NCH

    def planar(x):
        # [P, NCH, TCH, 4] -> [P, NCH, 4, TCH]
        return (np.asarray(x[sl]).reshape(P, NCH, tch, 4) != 0).transpose(0, 1, 3, 2)

    ab = np.stack([planar(A), planar(B)], axis=2)  # [P, NCH, 2, 4, TCH]
    ab = np.ascontiguousarray(ab).reshape(P, 8 * mpp)
    return (ab.astype(np.uint8) * FP8_ONE).view(ml_dtypes.float8_e4m3)


def kernel(A, B, trace=False):
    from concourse import bass_utils

    A = np.asarray(A)
    B = np.asarray(B)
    assert A.shape == (N_ROWS, 4) and B.shape == (N_ROWS, 4), (A.shape, B.shape)

    in_maps = []
    for i in range(N_CORES):
        sl = slice(i * R, (i + 1) * R)
        in_maps.append({"AB": _pack_ab(A, B, sl)})

    nc = _get_nc()
    res = bass_utils.run_bass_kernel_spmd(
        nc, in_maps, core_ids=list(range(N_CORES)), trace=trace,
    )
    _CACHE["last_results"] = res

    gt = np.empty((N_ROWS,), dtype=np.float32)
    eq = np.empty((N_ROWS,), dtype=np.float32)
    for i in range(N_CORES):
        o = np.asarray(res.results[i]["out"]).view(np.uint8).reshape(R)
        sl = slice(i * R, (i + 1) * R)
        gt[sl] = (o == FP8_ONE)
        eq[sl] = ((o & 0x7F) == 0)
    return gt.reshape(N_ROWS, 1), eq.reshape(N_ROWS, 1)
